# revision 36
# baseline (speedup 1.0000x reference)
"""Trainium2 Bass kernel: multi-head attention (B=2, N=2048, DIM=768, H=12, Dh=64),
sharded (batch x head-group) across 8 NeuronCores. Self-contained.

Per-core shard (core = b*4 + g, g in 0..3, heads 3g..3g+2):
  - computes Q^T,K^T (features on partitions) and V (tokens on partitions) from x[b]^T
  - scores S^T[k,q] per head via row-tiled K=64 matmuls (2 concurrent per slot via
    tile_position partition halves; head 2 pairs its own even/odd k-tiles through a
    partition-swapped copy of Q2/K2)
  - exp on ScalarE over [128,1024] PSUM tiles shared by the head pair
  - O~^T and softmax sums in one matmul: V is augmented with 64 ones-columns so
    rows 64:128 of the accumulator hold the sums broadcast across partitions
  - normalize straight out of PSUM (no evac copy): bit-trick Newton reciprocal
    on the sums half + final TT reading the o~ half from PSUM; the very last
    chain runs Ln->Exp on the then-idle ScalarE instead,
    project with this group's proj_w rows, partial out [2048, 768] f32
Host: shards inputs (bf16, scale folded into Wq, layouts pre-arranged), gathers:
  out[b] = -(sum_g partial_gb) + (qkv_b[v-part] @ proj_w + proj_b).
Scheduling: short PE warmup spam against HAM cold-clock, input DMAs fanned over
all three DMA queues (SP/ACT HWDGE + Pool SWDGE), first score step emitted right
after mt0/mt1 so the exp stream starts before the v-tiles, QK/V phase interleaved
with attention(qc=0), projection delayed one chunk, h0/h1 reciprocal chains under
the h2 phase (evacs BEFORE proj evictions on the DVE queue — chains after proj),
tail projection on the s2 ring + accumulator banks with o_a matmuls pre-started
under the last rec chain, ScalarE/VectorE split evictions, per-qt output DMAs
ending on the fast-draining HWDGE queues, and a trimmed TileContext teardown
(single-shot NEFF).
"""

import sys

for _p in ("/opt/trn_rl_repo",):
    if _p not in sys.path:
        sys.path.append(_p)

import numpy as np
import ml_dtypes

import concourse.bass as bass
import concourse.mybir as mybir
import concourse.tile as tile
from concourse.bass_utils import run_bass_kernel_spmd

BF16 = mybir.dt.bfloat16
F32 = mybir.dt.float32
bf16 = ml_dtypes.bfloat16

B, N, DIM = 2, 2048, 768
H, Dh = 12, 64
G = 3  # heads per core
NCORES = 8
QC = 512  # query chunk (free dim of score matmuls)
NQC = N // QC
KT = 128  # key tile (partition dim of S^T)
NKT = N // KT

# NOTE on exp: all exp runs on ScalarE. ACT throughput on [128,1024] tiles is
# ~853ns/step (the +352cyc in the latency formula is pipeline fill, not
# occupancy), which matches the ~861ns PE step, so the steady state is
# co-bound and offloading exp to the VectorE (tried: per-step Schraudolph
# int16 bf16-bits, half-tile splits, alternating steps) only adds cross-engine
# sync stalls (+4us/qc measured on HW for every variant).


# --------------------------------------------------------------------------
# workaround: this container's walrus accepts only ONE sync-wait per
# instruction ("Too many sync wait commands"). Split multi-wait sync_infos
# onto same-engine NoOps inserted right before the instruction.
def _patch_to_json():
    import orjson

    if getattr(bass.Bass, "_ant_json_patched", False):
        return
    orig = bass.Bass.to_json_bytes

    def to_json_bytes(self, *a, **kw):
        m = orjson.loads(orig(self, *a, **kw))

        def walk(o):
            if isinstance(o, dict):
                insts = o.get("instructions")
                if isinstance(insts, list) and insts and isinstance(insts[0], dict):
                    new = []
                    for inst in insts:
                        si = inst.get("sync_info")
                        waits = (si or {}).get("on_wait") or []
                        if len(waits) > 1:
                            for i, w in enumerate(waits[:-1]):
                                new.append(
                                    {
                                        "debug": inst.get("debug", 0),
                                        "engine": inst["engine"],
                                        "ins": [],
                                        "name": f"{inst['name']}-sw{i}",
                                        "opcode": "NoOp",
                                        "outs": [],
                                        "sync_info": {
                                            "on_update": [],
                                            "on_wait": [w],
                                        },
                                    }
                                )
                            si["on_wait"] = waits[-1:]
                        new.append(inst)
                    o["instructions"] = new
                for v in o.values():
                    walk(v)
            elif isinstance(o, list):
                for v in o:
                    walk(v)

        walk(m)
        return orjson.dumps(m)

    bass.Bass.to_json_bytes = to_json_bytes
    bass.Bass._ant_json_patched = True


# workaround: this container's walrus allows only 1 sync-wait on SP CTRL ops;
# Tile's kernel-tail drain piles every outstanding proc wait onto one Drain.
def _patch_tile_drain():
    from concourse.tile import TileContext, ScopedClock

    if getattr(TileContext, "_ant_drain_patched", False):
        return

    def _drain_and_barrier(self, tick_clock, wait_clock):
        nc = self.nc
        collector = nc.sync.nop(nofuse=True)
        wait_clock.add_sem_waits(
            collector.ins, ScopedClock({None: tick_clock.global_clock})
        )
        si = collector.ins.sync_info
        waits = list(si.on_wait) if si is not None else []
        if len(waits) > 1:
            si.on_wait = waits[:1]
            for w in waits[1:]:
                extra = nc.sync.nop(nofuse=True)
                extra.ins.sync_info = mybir.SyncInfo(on_wait=[w], on_update=[])
        nc.sync.drain()
        nc.all_engine_barrier()
        assert self.sems is not None
        popped = nc._tile_sem_poison_stack.pop()
        assert popped is self._sem_poison
        # The stock tail also does clear_and_free_semaphores + a second
        # all_engine_barrier (~4us of teardown). This kernel is single-shot
        # per NEFF load and allocates no sems after the tile scope, so skip
        # both (keep the bookkeeping pop above).

    TileContext._drain_and_barrier = _drain_and_barrier
    TileContext._ant_drain_patched = True


# --------------------------------------------------------------------------
def build_kernel():
    _patch_to_json()
    _patch_tile_drain()
    Exp = mybir.ActivationFunctionType.Exp
    Alu = mybir.AluOpType

    nc = bass.Bass(trn_type="TRN2")
    xT = nc.dram_tensor("xT", [DIM, N], BF16, kind="ExternalInput")
    wqk = nc.dram_tensor("wqk", [DIM, 384], BF16, kind="ExternalInput")
    bqk = nc.dram_tensor("bqk", [384], F32, kind="ExternalInput")
    wv = nc.dram_tensor("wv", [DIM, 192], BF16, kind="ExternalInput")
    wp = nc.dram_tensor("wp", [192, DIM], BF16, kind="ExternalInput")
    # partials ship as bf16: the host sums 4 partial head-groups per batch in
    # f64, so the extra quantization is ~0.2% while halving output DMA bytes
    out = nc.dram_tensor("out", [N, DIM], BF16, kind="ExternalOutput")

    KC = DIM // 128  # 6 contraction chunks

    with tile.TileContext(nc) as tc:
        with (
            tc.tile_pool(name="persist", bufs=1) as pp,
            tc.tile_pool(name="pt_act", bufs=4) as pta,
            tc.tile_pool(name="scratch", bufs=4) as sp,
            tc.tile_pool(name="osb", bufs=3) as op_,
            tc.tile_pool(name="ysb", bufs=4) as yp,
            tc.tile_pool(name="ps", bufs=3, space="PSUM") as ps,
            tc.tile_pool(name="ps_acc", bufs=2, space="PSUM") as ps_acc,
        ):
            # ---- persistent SBUF ----
            xT_sb = pp.tile([128, KC, N], BF16, tag="xT")
            wqk_sb = pp.tile([128, KC, 384], BF16, tag="wqk")
            wv_sb = pp.tile([128, KC, 192], BF16, tag="wv")
            wp_sb = pp.tile([128, 2, DIM], BF16, tag="wp")
            bqk_sb = pp.tile([128, 3], F32, tag="bqk")
            actbias_sb = pp.tile([128, 1], F32, tag="actbias")
            warm_sb = pp.tile([128, 8], BF16, tag="warm")
            qk_sb = pp.tile([128, 4, N], BF16, tag="qkT")  # mt: [Q0|Q1],[K0|K1],[Q2|K2],[K2d|Q2d]
            v_sb = pp.tile([128, NKT, 384], BF16, tag="vaug")  # per kt: 3x [v_h(64) | ones(64)]

            # PE clock (HAM) warmup on zeroed SBUF + early exp-table load,
            # all before the heavyweight DMAs and memsets are queued. Short
            # spam only: real matmuls start ~9us now (3-queue input DMA), so
            # the ramp continues on real work instead of delaying it.
            warm_in = pp.tile([128, 256], BF16, tag="warmmm")
            nc.vector.memset(warm_in[:], 0.0)
            nc.vector.memset(actbias_sb[:], 0.0)
            wps = ps.tile([128, 2 * QC], F32, tag="sa", name="warmps")[:, 0:256]
            for i in range(28):
                nc.tensor.matmul(wps[:], warm_in[:, 0:128], warm_in[:],
                                 start=(i == 0), stop=(i == 27))
            nc.scalar.activation(warm_sb[:], actbias_sb[:].to_broadcast((128, 8)), Exp)

            # Input DMAs fan out over all three DMA-capable queues (SP-HWDGE,
            # ACT-HWDGE, Pool-SWDGE). wqk is split by kc-halves across two
            # queues (same 768B lines, half the serial transfer) and the qq0
            # granules are balanced so no queue carries more than ~560KB
            # ahead of the first qk chain. The scalar queue only carries
            # early issues so it is clear well before the first exp.
            # (tried: wqk split by kc-halves across sync+scalar to halve its
            # serial transfer — the sliced AP transfers far slower than the
            # whole-tensor DMA and the first qk chain slipped 14.4->20.6us,
            # +9us on HW. Keep wqk whole on sync.)
            nc.sync.dma_start(wqk_sb[:], wqk.rearrange("(o p) m -> p o m", p=128))
            nc.scalar.dma_start(bqk_sb[:], bqk.rearrange("(m p) -> p m", p=128))
            for qq in range(NQC):
                for kc in range(KC):
                    if kc < 2:
                        eng = nc.scalar if qq < 2 else nc.sync
                    elif kc < 4:
                        eng = nc.gpsimd
                    else:
                        eng = nc.sync
                    eng.dma_start(
                        xT_sb[:, kc, QC * qq : QC * qq + QC],
                        xT[128 * kc : 128 * kc + 128, QC * qq : QC * qq + QC],
                    )
                if qq == 0:
                    nc.gpsimd.dma_start(wv_sb[:], wv.rearrange("(o p) m -> p o m", p=128))
                elif qq == 1:
                    nc.gpsimd.dma_start(wp_sb[:, 0, :], wp[0:128, :])
                    nc.gpsimd.dma_start(wp_sb[0:64, 1, :], wp[128:192, :])
            # only the ones-halves need the memset; v_tile writes the v-halves
            nc.vector.memset(
                v_sb[:].rearrange("p k (g c) -> p (k g) c", c=128)[:, :, 64:128], 1.0
            )

            def qk_mt(qc, mt):
                # Q^T / K^T projection, one mt slice of one 512-token chunk
                ps_t = ps.tile([128, 2 * QC], F32, tag="sa", name="qkps")[:, 0:QC]
                for kc in range(KC):
                    nc.tensor.matmul(
                        ps_t[:],
                        wqk_sb[:, kc, 128 * mt : 128 * mt + 128],
                        xT_sb[:, kc, QC * qc : QC * qc + QC],
                        start=(kc == 0),
                        stop=(kc == KC - 1),
                    )
                nc.vector.tensor_scalar(
                    qk_sb[:, mt, QC * qc : QC * qc + QC],
                    ps_t[:],
                    bqk_sb[:, mt : mt + 1],
                    None,
                    Alu.add,
                )

            def qk_swap(qc):
                sl = slice(QC * qc, QC * qc + QC)
                nc.sync.dma_start(qk_sb[0:64, 3, sl], qk_sb[64:128, 2, sl])
                nc.sync.dma_start(qk_sb[64:128, 3, sl], qk_sb[0:64, 2, sl])

            def v_tile(kt):
                    ps_t = ps.tile([128, 2 * QC], F32, tag="sa", name="vps")[:, 0:192]
                    for kc in range(KC):
                        nc.tensor.matmul(
                            ps_t[:],
                            xT_sb[:, kc, KT * kt : KT * kt + KT],
                            wv_sb[:, kc, :],
                            start=(kc == 0),
                            stop=(kc == KC - 1),
                        )
                    nc.vector.tensor_copy(
                        out=v_sb[:, kt, :].rearrange("p (h c) -> p h c", c=128)[:, :, 0:64],
                        in_=ps_t[:].rearrange("p (h c) -> p h c", c=64),
                    )

            # score matmul operands: heads 0/1 pair on partition halves; head 2
            # alternates halves by kt parity via the swapped copy in slot 3.
            def s_operands(h, kt):
                if h < 2:
                    po = 64 * h
                    return (1, po), (0, po)
                return ((3, 0) if kt % 2 == 0 else (2, 64)), ((2, 0) if kt % 2 == 0 else (3, 64))

            def s_mm(dst, h, kt, qc):
                (lm, lp), (rm, rp) = s_operands(h, kt)
                nc.tensor.matmul(
                    dst,
                    qk_sb[lp : lp + 64, lm, KT * kt : KT * kt + KT],
                    qk_sb[rp : rp + 64, rm, QC * qc : QC * qc + QC],
                    start=True,
                    stop=True,
                    tile_position=(lp, 0),
                )

            def attn_begin(qc):
                return {
                    "qc": qc,
                    "o_a": op_.tile([128, QC], BF16, tag="oa", name="oa"),
                    "o_b": op_.tile([64, QC], BF16, tag="ob", name="ob"),
                    "ocs": [],
                }

            def _pv(st, step, pt):
                for h, kt, off in step:
                    nc.tensor.matmul(
                        st["o_ps"][h][:],
                        v_sb[:, kt, 128 * h : 128 * h + 128],
                        pt[:, off : off + QC],
                        start=(kt == 0),
                        stop=(kt == NKT - 1),
                    )

            def attn_steps(st, heads, steps):
                qc = st["qc"]
                if "o_ps" not in st:
                    st["o_ps"] = {}
                o_ps = st["o_ps"]
                for h in heads:
                    if h not in o_ps:
                        o_ps[h] = ps_acc.tile([128, QC], F32, tag="acc", name="acc")
                for step in steps:
                    s2 = ps.tile([128, 2 * QC], F32, tag="sa", name="sa")
                    for h, kt, off in step:
                        s_mm(s2[:, off : off + QC], h, kt, qc)
                    pt = pta.tile([128, 2 * QC], BF16, tag="pta", name="pta")
                    nc.scalar.activation(pt[:], s2[:], Exp)
                    # PV is emitted one step late so the PE never waits on the
                    # exp of the step it just issued (exp pipelines one behind).
                    pend = st.setdefault("pend", [])
                    pend.append((step, pt))
                    if len(pend) > 1:
                        _pv(st, *pend.pop(0))

            MAGIC = 0x7EF311C3
            Ln = mybir.ActivationFunctionType.Ln

            def attn_evac(st, heads):
                # flush pending PV and evacuate the accumulators immediately:
                # the copies free the PSUM banks (the next phase's PV reuses
                # them) and must hit the DVE queue BEFORE the proj evictions
                # and the reciprocal chains (normalize() is emitted after the
                # proj so the yps-ring evictions are not stuck behind ~6us of
                # chain work — measured 5.9us/qc PE stall the other way).
                for p in st.get("pend") or []:
                    _pv(st, *p)
                st["pend"] = []
                for h in heads:
                    oc = sp.tile([128, QC], F32, tag="ocopy", name="ocopy")
                    nc.vector.tensor_copy(out=oc[:], in_=st["o_ps"][h][:])
                    del st["o_ps"][h]
                    st["ocs"].append((h, oc))

            def normalize(st):
                # rec = -(approx 1/sums): int bit-trick seed + 1 Newton step,
                # all on the VectorE; sign fixed on the host.
                o_a, o_b, ocs = st["o_a"], st["o_b"], st["ocs"]
                st["ocs"] = []
                for h, oc in ocs:
                    dst = o_a[64 * h : 64 * h + 64, :] if h < 2 else o_b[:]
                    seedt = sp.tile([128, QC], F32, tag="seed", name="seed")
                    seed = seedt[64:128, :]
                    nc.vector.tensor_scalar(
                        seed.bitcast(mybir.dt.int32),
                        oc[64:128, :].bitcast(mybir.dt.int32),
                        MAGIC, -1, Alu.subtract, Alu.mult,
                    )
                    ut = sp.tile([128, QC], F32, tag="nru", name="nru")
                    u = ut[64:128, :]
                    nc.vector.tensor_tensor(u, oc[64:128, :], seed, Alu.mult)
                    rect = sp.tile([128, QC], F32, tag="recip", name="recip")
                    rec = rect[0:64, :]
                    nc.vector.scalar_tensor_tensor(
                        rect[64:128, :], u, 2.0, seed, Alu.subtract, Alu.mult
                    )
                    nc.vector.tensor_copy(out=rec, in_=rect[64:128, :])
                    nc.vector.tensor_tensor(dst, oc[0:64, :], rec, Alu.mult)

            def finish_last(st):
                # the very last chain: ScalarE is out of exp work, so compute
                # rec = exp(-ln(sums)) there while the DVE does a negated
                # evac of the o~ half (TS frees the accumulator bank early so
                # the tail projection can pre-start into it); the final TT
                # composes them with the host-side sign convention.
                for p in st.get("pend") or []:
                    _pv(st, *p)
                st["pend"] = []
                (h,) = list(st["o_ps"])
                o_ps = st["o_ps"][h]
                lnt = sp.tile([128, QC], F32, tag="lnt", name="lnt")
                rect = sp.tile([128, QC], F32, tag="recip", name="recip")
                ocn = sp.tile([128, QC], F32, tag="ocopy", name="ocn")
                # (tried: two 256-col half-chains so qt0/qt1's o_b matmuls
                # unblock earlier — the per-half ACT pipeline fill costs more
                # than the earlier unblock saves: +8us on HW. Keep it whole.)
                nc.scalar.activation(lnt[0:64, :], o_ps[64:128, :], Ln)
                nc.vector.tensor_scalar(
                    ocn[0:64, :], o_ps[0:64, :], -1.0, None, Alu.mult
                )
                nc.scalar.activation(rect[0:64, :], lnt[0:64, :], Exp, scale=-1.0)
                nc.vector.tensor_tensor(
                    st["o_b"][:], ocn[0:64, :], rect[0:64, :], Alu.mult
                )
                del st["o_ps"][h]

            def proj_qt(qc, qt, o_a, o_b, act_evict=False, acc_pool=False):
                ys = yp.tile([128, DIM], BF16, tag="y", name="y")
                for nc2 in range(2):
                    nsl = slice(384 * nc2, 384 * nc2 + 384)
                    if acc_pool:
                        # tail proj: the attention accumulator banks are free,
                        # so don't touch the s2 ring
                        yps = ps_acc.tile([128, QC], F32, tag="acc", name="yps")[:, 0:384]
                    else:
                        yps = ps.tile([128, 2 * QC], F32, tag="sa", name="yps")[:, 0:384]
                    nc.tensor.matmul(
                        yps[:], o_a[:, 128 * qt : 128 * qt + 128], wp_sb[:, 0, nsl],
                        start=True, stop=False,
                    )
                    nc.tensor.matmul(
                        yps[:], o_b[:, 128 * qt : 128 * qt + 128], wp_sb[0:64, 1, nsl],
                        start=False, stop=True,
                    )
                    if act_evict and nc2 == 1:
                        # alternate engines in the tail so the 2-deep yps ring
                        # is not paced by one engine's eviction latency
                        nc.scalar.copy(ys[:, nsl], yps[:])
                    else:
                        nc.vector.tensor_copy(out=ys[:, nsl], in_=yps[:])
                    if acc_pool:
                        # tail: ship each half as soon as its eviction lands,
                        # rotating over all three DMA queues so the final
                        # drains overlap instead of serializing on one queue
                        eng = (nc.sync, nc.gpsimd, nc.scalar)[(2 * qt + nc2) % 3]
                        eng.dma_start(
                            out[QC * qc + 128 * qt : QC * qc + 128 * qt + 128, nsl],
                            ys[:, nsl],
                        )
                if not acc_pool:
                    nc.gpsimd.dma_start(
                        out[QC * qc + 128 * qt : QC * qc + 128 * qt + 128, :], ys[:]
                    )

            def proj(qc, o_a, o_b, act_evict=False, acc_pool=False):
                for qt in range(QC // 128):
                    proj_qt(qc, qt, o_a, o_b, act_evict, acc_pool)

            # tail output queues: gpsimd gets the EARLY halves (its SWDGE
            # drain is ~3us if a transfer is still in flight at teardown),
            # the last halves go to the HWDGE queues which drain fast.
            TAIL_ENG = ("gpsimd", "gpsimd", "gpsimd", "sync",
                        "sync", "scalar", "sync", "scalar")

            def tail_proj(qc, o_a, o_b):
                # runs on the s2/ps ring (free after the last exp): 3 tiles,
                # each holding BOTH nc2 halves, so qt0 AND qt1's o_a matmuls
                # pre-start under the last rec chain (they don't read o_b)
                # and each qt evicts with one strided copy instead of two.
                def oa_mms(qt, acc=False):
                    # acc: two 1-bank ps_acc tiles (freed early by the rec
                    # chain's TS) instead of one 2-bank ring tile — the ring
                    # slots are held by the last h2 s2 tiles until their
                    # exps drain, ~2us after the last PV
                    if acc:
                        t = [ps_acc.tile([128, QC], F32, tag="acc", name="yps")[:, 0:384]
                             for _ in range(2)]
                    else:
                        tt = ps.tile([128, 2 * QC], F32, tag="sa", name="yps")
                        t = [tt[:, 0:384], tt[:, 512:896]]
                    for nc2 in range(2):
                        nc.tensor.matmul(
                            t[nc2][:], o_a[:, 128 * qt : 128 * qt + 128],
                            wp_sb[:, 0, 384 * nc2 : 384 * nc2 + 384],
                            start=True, stop=False,
                        )
                    return t

                def finish_qt(qt, t, eng, act_evict):
                    ys = yp.tile([128, DIM], BF16, tag="y", name="y")
                    for nc2 in range(2):
                        nsl = slice(384 * nc2, 384 * nc2 + 384)
                        nc.tensor.matmul(
                            t[nc2][:], o_b[:, 128 * qt : 128 * qt + 128],
                            wp_sb[0:64, 1, nsl], start=False, stop=True,
                        )
                        evict = nc.scalar.copy if (act_evict and nc2 == 1) else (
                            lambda d, s: nc.vector.tensor_copy(out=d, in_=s))
                        evict(ys[:, nsl], t[nc2][:])
                    eng.dma_start(
                        out[QC * qc + 128 * qt : QC * qc + 128 * qt + 128, :],
                        ys[:],
                    )

                t1_ = oa_mms(1)          # ring slot frees first
                t0_ = oa_mms(0, acc=True)
                finish_qt(1, t1_, nc.gpsimd, False)
                finish_qt(0, t0_, nc.sync, True)
                finish_qt(2, oa_mms(2), nc.sync, False)
                finish_qt(3, oa_mms(3), nc.scalar, True)

            # ---- software-pipelined schedule ----
            # Phase 1 (QK/V projections) is interleaved with attention(qc=0):
            # attention consumes K^T/V k-tiles in order, and k-tile group g
            # becomes available right after its qk/v slices. The first score
            # step is emitted straight after mt0/mt1 so the exp stream starts
            # before the v-tiles and the mt2 chain (PV is deferred one step,
            # so the v-tile only has to be emitted before the NEXT step).
            def pair_steps(kts):
                return [((0, kt, 0), (1, kt, QC)) for kt in kts]

            def h2_steps(irange):
                return [((2, 2 * i, 0), (2, 2 * i + 1, QC)) for i in irange]

            qk_mt(0, 0)
            qk_mt(0, 1)
            st0 = attn_begin(0)
            attn_steps(st0, (0, 1), pair_steps([0]))
            for kt in range(0, 4):
                v_tile(kt)
            attn_steps(st0, (0, 1), pair_steps(range(1, 4)))
            qk_mt(0, 2)
            qk_swap(0)
            for qq in range(1, NQC):
                qk_mt(qq, 0)
                qk_mt(qq, 1)
                for kt in range(4 * qq, 4 * qq + 4):
                    v_tile(kt)
                attn_steps(st0, (0, 1), pair_steps(range(4 * qq, 4 * qq + 4)))
                qk_mt(qq, 2)
                qk_swap(qq)
            attn_evac(st0, (0, 1))
            normalize(st0)  # h0/h1 chains run under the h2 phase
            attn_steps(st0, (2,), h2_steps(range(NKT // 2)))
            attn_evac(st0, (2,))
            normalize(st0)
            prev = st0
            for qc in range(1, NQC):
                st = attn_begin(qc)
                attn_steps(st, (0, 1), pair_steps(range(NKT)))
                attn_evac(st, (0, 1))
                proj(qc - 1, prev["o_a"], prev["o_b"])  # yps ring drains during head-2
                normalize(st)  # h0/h1 chains run under the h2 phase
                attn_steps(st, (2,), h2_steps(range(NKT // 2)))
                if qc == NQC - 1:
                    finish_last(st)
                else:
                    attn_evac(st, (2,))
                    normalize(st)
                prev = st
            # tail: only the last h2 rec chain remains ahead of this
            tail_proj(NQC - 1, prev["o_a"], prev["o_b"])
    return nc


_NC_CACHE = {}


def _get_nc():
    if "nc" not in _NC_CACHE:
        _NC_CACHE["nc"] = build_kernel()
    return _NC_CACHE["nc"]


def kernel(x, qkv_w, qkv_b, proj_w, proj_b):
    x = np.asarray(x, np.float32)
    qkv_w = np.asarray(qkv_w, np.float32)
    qkv_b = np.asarray(qkv_b, np.float32)
    proj_w = np.asarray(proj_w, np.float32)
    proj_b = np.asarray(proj_b, np.float32)

    wr = qkv_w.reshape(DIM, 3, H, Dh)
    br = qkv_b.reshape(3, H, Dh)
    scale = Dh ** -0.5

    in_maps = []
    for core in range(NCORES):
        b, g = divmod(core, 4)
        hs = slice(G * g, G * g + G)
        wq = wr[:, 0, hs, :].reshape(DIM, G * Dh) * scale  # fold softmax scale into Q
        wk = wr[:, 1, hs, :].reshape(DIM, G * Dh)
        wvm = wr[:, 2, hs, :].reshape(DIM, G * Dh)
        bq = br[0, hs].reshape(G * Dh) * scale
        bk = br[1, hs].reshape(G * Dh)
        # column order: mt0=[Q0|Q1], mt1=[K0|K1], mt2=[Q2|K2] (64 cols per head)
        wqk_c = np.concatenate(
            [wq[:, 0:128], wk[:, 0:128], wq[:, 128:192], wk[:, 128:192]], axis=1
        )
        bqk_c = np.concatenate([bq[0:128], bk[0:128], bq[128:192], bk[128:192]])
        in_maps.append(
            {
                "xT": np.ascontiguousarray(x[b].T).astype(bf16),
                "wqk": np.ascontiguousarray(wqk_c).astype(bf16),
                "bqk": np.ascontiguousarray(bqk_c),
                "wv": np.ascontiguousarray(wvm).astype(bf16),
                "wp": np.ascontiguousarray(proj_w[64 * G * g : 64 * G * (g + 1), :]).astype(bf16),
            }
        )

    nc = _get_nc()
    res = run_bass_kernel_spmd(nc, in_maps, core_ids=list(range(NCORES)))
    _NC_CACHE["last_result"] = res

    bias_row = (br[2].reshape(DIM).astype(np.float64) @ proj_w.astype(np.float64)
                + proj_b.astype(np.float64)).astype(np.float32)
    out = np.zeros((B, N, DIM), np.float32)
    for b in range(B):
        acc = np.zeros((N, DIM), np.float64)
        for g in range(4):
            acc += res.results[4 * b + g]["out"].astype(np.float64)
        out[b] = (-acc).astype(np.float32) + bias_row
    return out

